# revision 23
# baseline (speedup 1.0000x reference)
import os
import sys
from contextlib import ExitStack

import numpy as np

if "/opt/trn_rl_repo" not in sys.path:
    sys.path.insert(0, "/opt/trn_rl_repo")

import concourse.bass as bass
import concourse.tile as tile
from concourse import bacc, mybir
from concourse.masks import make_identity

B, C, L = 4, 512, 2048
H, Ch = 8, 64
N_CORES = 8
KC = 4
NT = 4
LB = 2
MC = 16
F32 = mybir.dt.float32
F32R = mybir.dt.float32r
BF16 = mybir.dt.bfloat16

LAST_EXEC_NS = None


def _build_nc():
    nc = bacc.Bacc("TRN2", target_bir_lowering=False, debug=False,
                   num_devices=N_CORES)
    x_d = nc.dram_tensor("x", [2, C, L], F32, kind="ExternalInput").ap()
    w_d = nc.dram_tensor("w", [3, C, 128], F32, kind="ExternalInput").ap()
    b_d = nc.dram_tensor("b", [3, 128], F32, kind="ExternalInput").ap()
    out_d = nc.dram_tensor("out", [2, 2, L, Ch], F32, kind="ExternalOutput").ap()
    v_d = nc.dram_tensor("vdram", [2, 128, L], BF16).ap()

    with tile.TileContext(nc) as tc, ExitStack() as ctx:
        const = ctx.enter_context(tc.tile_pool(name="const", bufs=1))
        persist = ctx.enter_context(tc.tile_pool(name="persist", bufs=1))

        ident = const.tile([128, 128], F32, tag="ident", name="ident")
        make_identity(nc, ident[:])

        w_sb = const.tile([128, 3, KC, 128], F32R, tag="w", name="w")
        nc.sync.dma_start(w_sb[:], w_d.rearrange("g (kc p) m -> p g kc m",
                                                 p=128).bitcast(F32R))
        b_sb = const.tile([128, 3], F32, tag="b", name="b")
        nc.sync.dma_start(b_sb[:], b_d.rearrange("g p -> p g"))

        x_sb = {}
        for bi in range(2):
            x_sb[bi] = persist.tile([128, KC, L], F32R, tag=f"x{bi}",
                                    name=f"x{bi}")
            for kc in range(KC):
                nc.sync.dma_start(
                    x_sb[bi][:, kc, :],
                    x_d[bi, kc * 128:(kc + 1) * 128, :].bitcast(F32R))

        q_sb, k_sb, v_sb, v2_sb = {}, {}, {}, {}
        for bi in range(2):
            q_sb[bi] = persist.tile([128, L], BF16, tag=f"q{bi}", name=f"q{bi}")
            v_sb[bi] = persist.tile([128, L], BF16, tag=f"v{bi}", name=f"v{bi}")
            for hi in range(2):
                t = persist.tile([128, L], BF16, tag=f"k{bi}{hi}",
                                 name=f"k{bi}{hi}")
                nc.vector.memset(t[(1 - hi) * 64:(2 - hi) * 64, :], 0.0)
                k_sb[bi, hi] = t
        for bi in range(2):
            for hi in range(2):
                v2_sb[bi, hi] = persist.tile([128, MC, Ch + 1], BF16,
                                             tag=f"v2_{bi}{hi}",
                                             name=f"v2_{bi}{hi}")
                nc.vector.memset(v2_sb[bi, hi][:, :, Ch:Ch + 1], 1.0)

        proj_ps = ctx.enter_context(
            tc.tile_pool(name="proj_ps", bufs=2, space="PSUM"))

        def bias_copy(eng, dst_ap, src_ap, bias_ap):
            if eng == 0:
                nc.vector.tensor_scalar_add(dst_ap, src_ap, bias_ap)
            else:
                nc.scalar.add(dst_ap, src_ap, bias_ap)

        dst = [q_sb, None, v_sb]
        for bi in range(2):
            for g in (2, 1, 0):
                for half in range(2):
                    ps = [proj_ps.tile([128, 512], F32, tag="pp", name="pp")
                          for _ in range(2)]
                    for kc in range(KC):
                        for j in range(2):
                            nt = half * 2 + j
                            nc.tensor.matmul(
                                ps[j][:],
                                w_sb[:, g, kc, :],
                                x_sb[bi][:, kc, nt * 512:(nt + 1) * 512],
                                start=(kc == 0), stop=(kc == KC - 1))
                    for j in range(2):
                        nt = half * 2 + j
                        sl = slice(nt * 512, (nt + 1) * 512)
                        if g == 1:
                            for hi in range(2):
                                pr = slice(hi * 64, (hi + 1) * 64)
                                bias_copy(hi, k_sb[bi, hi][pr, sl],
                                          ps[j][pr, :], b_sb[pr, g:g + 1])
                        else:
                            bias_copy(j, dst[g][bi][:, sl], ps[j][:],
                                      b_sb[:, g:g + 1])
                if g == 2:
                    nc.sync.dma_start(v_d[bi], v_sb[bi][:])
                    for hi in range(2):
                        for mc in range(MC):
                            vsrc = v_d[bi,
                                       hi * 64 + 4 * mc:hi * 64 + 4 * mc + 4,
                                       :]
                            nc.sync.dma_start(
                                v2_sb[bi, hi][:, mc, 0:Ch],
                                vsrc.rearrange("a (j cc) -> (a j) cc",
                                               j=32, cc=Ch))

        s_pool = ctx.enter_context(
            tc.tile_pool(name="s_ps", bufs=2, space="PSUM"))
        av_pool = ctx.enter_context(
            tc.tile_pool(name="av_ps", bufs=1, space="PSUM"))
        pt_pool = ctx.enter_context(tc.tile_pool(name="pt", bufs=4))
        avs_pool = ctx.enter_context(tc.tile_pool(name="avs", bufs=2))
        rcp_pool = ctx.enter_context(tc.tile_pool(name="rcp", bufs=4))
        o_pool = ctx.enter_context(tc.tile_pool(name="o", bufs=4))

        def heat(n):
            for _ in range(n):
                ht = av_pool.tile([128, 512], F32, tag="av", name="ht")
                nc.tensor.matmul(ht[:], v_sb[0][:, 0:128], v_sb[0][:, 0:512],
                                 start=True, stop=True)

        first = True
        for bi in range(2):
            for hi in range(2):
                for lb in range(LB):
                    l0 = lb * 1024
                    heat(20 if first else 2)
                    first = False
                    av = av_pool.tile([Ch + 1, 1024], F32, tag="av", name="av")
                    for mc in range(MC):
                        s = s_pool.tile([128, 1024], F32, tag="s", name="s")
                        for n2 in range(2):
                            nc.tensor.matmul(
                                s[:, n2 * 512:(n2 + 1) * 512],
                                k_sb[bi, hi][:, mc * 128:(mc + 1) * 128],
                                q_sb[bi][:, l0 + n2 * 512:l0 + (n2 + 1) * 512],
                                start=True, stop=True)
                        pt = pt_pool.tile([128, 1024], BF16, tag="pt",
                                          name="pt")
                        nc.scalar.activation(pt[:], s[:],
                                             mybir.ActivationFunctionType.Exp,
                                             scale=0.125)
                        for n2 in range(2):
                            nc.tensor.matmul(
                                av[:, n2 * 512:(n2 + 1) * 512],
                                v2_sb[bi, hi][:, mc, :],
                                pt[:, n2 * 512:(n2 + 1) * 512],
                                start=(mc == 0), stop=(mc == MC - 1))
                    avs = avs_pool.tile([Ch + 1, 1024], F32, tag="avs",
                                        name="avs")
                    nc.vector.tensor_copy(avs[:], av[:])
                    for jj in range(8):
                        tp = s_pool.tile([128, Ch + 1], F32, tag="s",
                                         name="tp")
                        nc.tensor.transpose(
                            tp[:], avs[:, jj * 128:(jj + 1) * 128],
                            ident[0:Ch + 1, 0:Ch + 1])
                        rcp = rcp_pool.tile([128, 1], F32, tag="rcp",
                                            name="rcp")
                        nc.vector.reciprocal(rcp[:], tp[:, Ch:Ch + 1])
                        o = o_pool.tile([128, Ch], F32, tag="o", name="o")
                        nc.vector.tensor_scalar_mul(o[:], tp[:, 0:Ch], rcp[:])
                        nc.sync.dma_start(
                            out_d[bi, hi, l0 + jj * 128:l0 + (jj + 1) * 128, :],
                            o[:])
    nc.compile()
    return nc


_NC_CACHE = None


def _get_nc():
    global _NC_CACHE
    if _NC_CACHE is None:
        _NC_CACHE = _build_nc()
    return _NC_CACHE


def _make_in_maps(x, wq, bq, wk, bk, wv, bv):
    in_maps = []
    for core in range(N_CORES):
        bg, hg = divmod(core, 4)
        bs = [2 * bg, 2 * bg + 1]
        hs = [2 * hg, 2 * hg + 1]

        def packw(w):
            return np.concatenate(
                [w[h * Ch:(h + 1) * Ch].T for h in hs], axis=1)

        def packb(b):
            return np.concatenate([b[h * Ch:(h + 1) * Ch] for h in hs])

        in_maps.append({
            "x": np.ascontiguousarray(x[bs]),
            "w": np.ascontiguousarray(
                np.stack([packw(wq), packw(wk), packw(wv)])),
            "b": np.ascontiguousarray(
                np.stack([packb(bq), packb(bk), packb(bv)])),
        })
    return in_maps


def kernel(x, wq, bq, wk, bk, wv, bv):
    global LAST_EXEC_NS
    from concourse.bass_utils import run_bass_kernel_spmd

    nc = _get_nc()
    in_maps = _make_in_maps(
        np.asarray(x, dtype=np.float32),
        np.asarray(wq, np.float32), np.asarray(bq, np.float32),
        np.asarray(wk, np.float32), np.asarray(bk, np.float32),
        np.asarray(wv, np.float32), np.asarray(bv, np.float32))

    trace = os.environ.get("BASS_KERNEL_TRACE", "0") == "1"
    kwargs = {}
    if trace:
        kwargs.update(trace=True, trace_cores=[0])
    res = run_bass_kernel_spmd(nc, in_maps, list(range(N_CORES)), **kwargs)
    LAST_EXEC_NS = res.exec_time_ns

    out = np.empty((B, C, L), dtype=np.float32)
    for core in range(N_CORES):
        bg, hg = divmod(core, 4)
        o = res.results[core]["out"]
        for bi in range(2):
            for hi in range(2):
                b_ = 2 * bg + bi
                h_ = 2 * hg + hi
                out[b_, h_ * Ch:(h_ + 1) * Ch, :] = o[bi, hi].reshape(Ch, L)
    return out


# revision 24
# speedup vs baseline: 1.2257x; 1.2257x over previous
import os
import sys
from contextlib import ExitStack

import numpy as np

if "/opt/trn_rl_repo" not in sys.path:
    sys.path.insert(0, "/opt/trn_rl_repo")

import concourse.bass as bass
import concourse.tile as tile
from concourse import bacc, mybir
from concourse.masks import make_identity

B, C, L = 4, 512, 2048
H, Ch = 8, 64
N_CORES = 8
KC = 4
NT = 4
LB = 2
MC = 16
F32 = mybir.dt.float32
F32R = mybir.dt.float32r
BF16 = mybir.dt.bfloat16

LAST_EXEC_NS = None


def _build_nc():
    nc = bacc.Bacc("TRN2", target_bir_lowering=False, debug=False,
                   num_devices=N_CORES)
    x_d = nc.dram_tensor("x", [2, C, L], F32, kind="ExternalInput").ap()
    w_d = nc.dram_tensor("w", [3, C, 128], F32, kind="ExternalInput").ap()
    b_d = nc.dram_tensor("b", [3, 128], F32, kind="ExternalInput").ap()
    out_d = nc.dram_tensor("out", [2, 2, L, Ch], F32, kind="ExternalOutput").ap()
    v_d = nc.dram_tensor("vdram", [2, 128, L], BF16).ap()

    with tile.TileContext(nc) as tc, ExitStack() as ctx:
        const = ctx.enter_context(tc.tile_pool(name="const", bufs=1))
        persist = ctx.enter_context(tc.tile_pool(name="persist", bufs=1))

        ident = const.tile([128, 128], F32, tag="ident", name="ident")
        make_identity(nc, ident[:])

        w_sb = const.tile([128, 3, KC, 128], F32R, tag="w", name="w")
        nc.sync.dma_start(w_sb[:], w_d.rearrange("g (kc p) m -> p g kc m",
                                                 p=128).bitcast(F32R))
        b_sb = const.tile([128, 3], F32, tag="b", name="b")
        nc.sync.dma_start(b_sb[:], b_d.rearrange("g p -> p g"))

        x_sb = {}
        for bi in range(2):
            x_sb[bi] = persist.tile([128, KC, L], F32R, tag=f"x{bi}",
                                    name=f"x{bi}")
            for kc in range(KC):
                nc.sync.dma_start(
                    x_sb[bi][:, kc, :],
                    x_d[bi, kc * 128:(kc + 1) * 128, :].bitcast(F32R))

        q_sb, k_sb, v_sb, v2_sb = {}, {}, {}, {}
        for bi in range(2):
            q_sb[bi] = persist.tile([128, L], BF16, tag=f"q{bi}", name=f"q{bi}")
            v_sb[bi] = persist.tile([128, L], BF16, tag=f"v{bi}", name=f"v{bi}")
            for hi in range(2):
                t = persist.tile([128, L], BF16, tag=f"k{bi}{hi}",
                                 name=f"k{bi}{hi}")
                nc.vector.memset(t[(1 - hi) * 64:(2 - hi) * 64, :], 0.0)
                k_sb[bi, hi] = t
        for bi in range(2):
            for hi in range(2):
                v2_sb[bi, hi] = persist.tile([128, MC, Ch + 1], BF16,
                                             tag=f"v2_{bi}{hi}",
                                             name=f"v2_{bi}{hi}")
                nc.vector.memset(v2_sb[bi, hi][:, :, Ch:Ch + 1], 1.0)

        proj_ps = ctx.enter_context(
            tc.tile_pool(name="proj_ps", bufs=2, space="PSUM"))

        def bias_copy(eng, dst_ap, src_ap, bias_ap):
            if eng == 0:
                nc.vector.tensor_scalar_add(dst_ap, src_ap, bias_ap)
            else:
                nc.scalar.add(dst_ap, src_ap, bias_ap)

        dst = [q_sb, None, v_sb]
        for bi in range(2):
            for g in (2, 1, 0):
                for half in range(2):
                    ps = [proj_ps.tile([128, 512], F32, tag="pp", name="pp")
                          for _ in range(2)]
                    for kc in range(KC):
                        for j in range(2):
                            nt = half * 2 + j
                            nc.tensor.matmul(
                                ps[j][:],
                                w_sb[:, g, kc, :],
                                x_sb[bi][:, kc, nt * 512:(nt + 1) * 512],
                                start=(kc == 0), stop=(kc == KC - 1))
                    for j in range(2):
                        nt = half * 2 + j
                        sl = slice(nt * 512, (nt + 1) * 512)
                        if g == 1:
                            for hi in range(2):
                                pr = slice(hi * 64, (hi + 1) * 64)
                                bias_copy(hi, k_sb[bi, hi][pr, sl],
                                          ps[j][pr, :], b_sb[pr, g:g + 1])
                        else:
                            bias_copy(j, dst[g][bi][:, sl], ps[j][:],
                                      b_sb[:, g:g + 1])
                if g == 2:
                    nc.sync.dma_start(v_d[bi], v_sb[bi][:])
                    for hi in range(2):
                        for mc in range(MC):
                            vsrc = v_d[bi,
                                       hi * 64 + 4 * mc:hi * 64 + 4 * mc + 4,
                                       :]
                            nc.sync.dma_start(
                                v2_sb[bi, hi][:, mc, 0:Ch],
                                vsrc.rearrange("a (j cc) -> (a j) cc",
                                               j=32, cc=Ch))

        s_pool = ctx.enter_context(
            tc.tile_pool(name="s_ps", bufs=2, space="PSUM"))
        av_pool = ctx.enter_context(
            tc.tile_pool(name="av_ps", bufs=1, space="PSUM"))
        pt_pool = ctx.enter_context(tc.tile_pool(name="pt", bufs=4))
        avs_pool = ctx.enter_context(tc.tile_pool(name="avs", bufs=2))
        rcp_pool = ctx.enter_context(tc.tile_pool(name="rcp", bufs=4))
        o_pool = ctx.enter_context(tc.tile_pool(name="o", bufs=4))

        def heat(n):
            for _ in range(n):
                ht = av_pool.tile([128, 512], F32, tag="av", name="ht")
                nc.tensor.matmul(ht[:], v_sb[0][:, 0:128], v_sb[0][:, 0:512],
                                 start=True, stop=True)

        first = True
        for bi in range(2):
            for hi in range(2):
                for lb in range(LB):
                    l0 = lb * 1024
                    heat(20 if first else 2)
                    first = False
                    av = av_pool.tile([Ch + 1, 1024], F32, tag="av", name="av")
                    for mc in range(MC):
                        s = s_pool.tile([128, 1024], F32, tag="s", name="s")
                        for n2 in range(2):
                            nc.tensor.matmul(
                                s[:, n2 * 512:(n2 + 1) * 512],
                                k_sb[bi, hi][:, mc * 128:(mc + 1) * 128],
                                q_sb[bi][:, l0 + n2 * 512:l0 + (n2 + 1) * 512],
                                start=True, stop=True)
                        pt = pt_pool.tile([128, 1024], BF16, tag="pt",
                                          name="pt")
                        nc.scalar.activation(pt[:], s[:],
                                             mybir.ActivationFunctionType.Exp,
                                             scale=0.125)
                        for n2 in range(2):
                            nc.tensor.matmul(
                                av[:, n2 * 512:(n2 + 1) * 512],
                                v2_sb[bi, hi][:, mc, :],
                                pt[:, n2 * 512:(n2 + 1) * 512],
                                start=(mc == 0), stop=(mc == MC - 1))
                    avs = avs_pool.tile([Ch + 1, 1024], F32, tag="avs",
                                        name="avs")
                    nc.vector.tensor_copy(avs[:], av[:])
                    for jj in range(8):
                        tp = proj_ps.tile([128, Ch + 1], F32, tag="pp",
                                          name="tp")
                        nc.tensor.transpose(
                            tp[:], avs[:, jj * 128:(jj + 1) * 128],
                            ident[0:Ch + 1, 0:Ch + 1])
                        rcp = rcp_pool.tile([128, 1], F32, tag="rcp",
                                            name="rcp")
                        nc.vector.reciprocal(rcp[:], tp[:, Ch:Ch + 1])
                        o = o_pool.tile([128, Ch], F32, tag="o", name="o")
                        nc.vector.tensor_scalar_mul(o[:], tp[:, 0:Ch], rcp[:])
                        nc.sync.dma_start(
                            out_d[bi, hi, l0 + jj * 128:l0 + (jj + 1) * 128, :],
                            o[:])
    nc.compile()
    return nc


_NC_CACHE = None


def _get_nc():
    global _NC_CACHE
    if _NC_CACHE is None:
        _NC_CACHE = _build_nc()
    return _NC_CACHE


def _make_in_maps(x, wq, bq, wk, bk, wv, bv):
    in_maps = []
    for core in range(N_CORES):
        bg, hg = divmod(core, 4)
        bs = [2 * bg, 2 * bg + 1]
        hs = [2 * hg, 2 * hg + 1]

        def packw(w):
            return np.concatenate(
                [w[h * Ch:(h + 1) * Ch].T for h in hs], axis=1)

        def packb(b):
            return np.concatenate([b[h * Ch:(h + 1) * Ch] for h in hs])

        in_maps.append({
            "x": np.ascontiguousarray(x[bs]),
            "w": np.ascontiguousarray(
                np.stack([packw(wq), packw(wk), packw(wv)])),
            "b": np.ascontiguousarray(
                np.stack([packb(bq), packb(bk), packb(bv)])),
        })
    return in_maps


def kernel(x, wq, bq, wk, bk, wv, bv):
    global LAST_EXEC_NS
    from concourse.bass_utils import run_bass_kernel_spmd

    nc = _get_nc()
    in_maps = _make_in_maps(
        np.asarray(x, dtype=np.float32),
        np.asarray(wq, np.float32), np.asarray(bq, np.float32),
        np.asarray(wk, np.float32), np.asarray(bk, np.float32),
        np.asarray(wv, np.float32), np.asarray(bv, np.float32))

    trace = os.environ.get("BASS_KERNEL_TRACE", "0") == "1"
    kwargs = {}
    if trace:
        kwargs.update(trace=True, trace_cores=[0])
    res = run_bass_kernel_spmd(nc, in_maps, list(range(N_CORES)), **kwargs)
    LAST_EXEC_NS = res.exec_time_ns

    out = np.empty((B, C, L), dtype=np.float32)
    for core in range(N_CORES):
        bg, hg = divmod(core, 4)
        o = res.results[core]["out"]
        for bi in range(2):
            for hi in range(2):
                b_ = 2 * bg + bi
                h_ = 2 * hg + hi
                out[b_, h_ * Ch:(h_ + 1) * Ch, :] = o[bi, hi].reshape(Ch, L)
    return out


# revision 25
# speedup vs baseline: 1.2811x; 1.0452x over previous
import os
import sys
from contextlib import ExitStack

import numpy as np

if "/opt/trn_rl_repo" not in sys.path:
    sys.path.insert(0, "/opt/trn_rl_repo")

import concourse.bass as bass
import concourse.tile as tile
from concourse import bacc, mybir
from concourse.masks import make_identity

B, C, L = 4, 512, 2048
H, Ch = 8, 64
N_CORES = 8
KC = 4
NT = 4
LB = 2
MC = 16
F32 = mybir.dt.float32
F32R = mybir.dt.float32r
BF16 = mybir.dt.bfloat16

LAST_EXEC_NS = None


def _build_nc():
    nc = bacc.Bacc("TRN2", target_bir_lowering=False, debug=False,
                   num_devices=N_CORES)
    x_d = nc.dram_tensor("x", [2, C, L], F32, kind="ExternalInput").ap()
    w_d = nc.dram_tensor("w", [3, C, 128], F32, kind="ExternalInput").ap()
    b_d = nc.dram_tensor("b", [3, 128], F32, kind="ExternalInput").ap()
    out_d = nc.dram_tensor("out", [2, 2, L, Ch], F32, kind="ExternalOutput").ap()
    v_d = nc.dram_tensor("vdram", [2, 128, L], BF16).ap()

    with tile.TileContext(nc) as tc, ExitStack() as ctx:
        const = ctx.enter_context(tc.tile_pool(name="const", bufs=1))
        persist = ctx.enter_context(tc.tile_pool(name="persist", bufs=1))

        ident = const.tile([128, 128], F32, tag="ident", name="ident")
        make_identity(nc, ident[:])

        w_sb = const.tile([128, 3, KC, 128], F32R, tag="w", name="w")
        nc.sync.dma_start(w_sb[:], w_d.rearrange("g (kc p) m -> p g kc m",
                                                 p=128).bitcast(F32R))
        b_sb = const.tile([128, 3], F32, tag="b", name="b")
        nc.sync.dma_start(b_sb[:], b_d.rearrange("g p -> p g"))

        x_sb = {}
        for bi in range(2):
            x_sb[bi] = persist.tile([128, KC, L], F32R, tag=f"x{bi}",
                                    name=f"x{bi}")
            for kc in range(KC):
                nc.sync.dma_start(
                    x_sb[bi][:, kc, :],
                    x_d[bi, kc * 128:(kc + 1) * 128, :].bitcast(F32R))

        q_sb, k_sb, v_sb, v2_sb = {}, {}, {}, {}
        for bi in range(2):
            q_sb[bi] = persist.tile([128, L], BF16, tag=f"q{bi}", name=f"q{bi}")
            v_sb[bi] = persist.tile([128, L], BF16, tag=f"v{bi}", name=f"v{bi}")
            for hi in range(2):
                t = persist.tile([128, L], BF16, tag=f"k{bi}{hi}",
                                 name=f"k{bi}{hi}")
                nc.vector.memset(t[(1 - hi) * 64:(2 - hi) * 64, :], 0.0)
                k_sb[bi, hi] = t
        for bi in range(2):
            for hi in range(2):
                v2_sb[bi, hi] = persist.tile([128, MC, Ch + 1], BF16,
                                             tag=f"v2_{bi}{hi}",
                                             name=f"v2_{bi}{hi}")
                nc.vector.memset(v2_sb[bi, hi][:, :, Ch:Ch + 1], 1.0)

        proj_ps = ctx.enter_context(
            tc.tile_pool(name="proj_ps", bufs=2, space="PSUM"))

        def bias_copy(eng, dst_ap, src_ap, bias_ap):
            if eng == 0:
                nc.vector.tensor_scalar_add(dst_ap, src_ap, bias_ap)
            else:
                nc.scalar.add(dst_ap, src_ap, bias_ap)

        dst = [q_sb, None, v_sb]
        for bi in range(2):
            for g in (2, 1, 0):
                for half in range(2):
                    ps = [proj_ps.tile([128, 512], F32, tag="pp", name="pp")
                          for _ in range(2)]
                    for kc in range(KC):
                        for j in range(2):
                            nt = half * 2 + j
                            nc.tensor.matmul(
                                ps[j][:],
                                w_sb[:, g, kc, :],
                                x_sb[bi][:, kc, nt * 512:(nt + 1) * 512],
                                start=(kc == 0), stop=(kc == KC - 1))
                    for j in range(2):
                        nt = half * 2 + j
                        sl = slice(nt * 512, (nt + 1) * 512)
                        if g == 1:
                            for hi in range(2):
                                pr = slice(hi * 64, (hi + 1) * 64)
                                bias_copy(hi, k_sb[bi, hi][pr, sl],
                                          ps[j][pr, :], b_sb[pr, g:g + 1])
                        else:
                            bias_copy(j, dst[g][bi][:, sl], ps[j][:],
                                      b_sb[:, g:g + 1])
                if g == 2:
                    nc.sync.dma_start(v_d[bi], v_sb[bi][:])
                    for hi in range(2):
                        for mc in range(MC):
                            vsrc = v_d[bi,
                                       hi * 64 + 4 * mc:hi * 64 + 4 * mc + 4,
                                       :]
                            nc.sync.dma_start(
                                v2_sb[bi, hi][:, mc, 0:Ch],
                                vsrc.rearrange("a (j cc) -> (a j) cc",
                                               j=32, cc=Ch))

        s_pool = ctx.enter_context(
            tc.tile_pool(name="s_ps", bufs=2, space="PSUM"))
        av_pool = ctx.enter_context(
            tc.tile_pool(name="av_ps", bufs=1, space="PSUM"))
        pt_pool = ctx.enter_context(tc.tile_pool(name="pt", bufs=4))
        avs_pool = ctx.enter_context(tc.tile_pool(name="avs", bufs=2))
        rcp_pool = ctx.enter_context(tc.tile_pool(name="rcp", bufs=4))
        o_pool = ctx.enter_context(tc.tile_pool(name="o", bufs=4))

        for bi in range(2):
            for hi in range(2):
                for lb in range(LB):
                    l0 = lb * 1024
                    av = av_pool.tile([Ch + 1, 1024], F32, tag="av", name="av")
                    for mc in range(MC):
                        s = s_pool.tile([128, 1024], F32, tag="s", name="s")
                        for n2 in range(2):
                            nc.tensor.matmul(
                                s[:, n2 * 512:(n2 + 1) * 512],
                                k_sb[bi, hi][:, mc * 128:(mc + 1) * 128],
                                q_sb[bi][:, l0 + n2 * 512:l0 + (n2 + 1) * 512],
                                start=True, stop=True)
                        pt = pt_pool.tile([128, 1024], BF16, tag="pt",
                                          name="pt")
                        nc.scalar.activation(pt[:], s[:],
                                             mybir.ActivationFunctionType.Exp,
                                             scale=0.125)
                        for n2 in range(2):
                            nc.tensor.matmul(
                                av[:, n2 * 512:(n2 + 1) * 512],
                                v2_sb[bi, hi][:, mc, :],
                                pt[:, n2 * 512:(n2 + 1) * 512],
                                start=(mc == 0), stop=(mc == MC - 1))
                    avs = avs_pool.tile([Ch + 1, 1024], F32, tag="avs",
                                        name="avs")
                    nc.vector.tensor_copy(avs[:], av[:])
                    for jj in range(8):
                        tp = proj_ps.tile([128, Ch + 1], F32, tag="pp",
                                          name="tp")
                        nc.tensor.transpose(
                            tp[:], avs[:, jj * 128:(jj + 1) * 128],
                            ident[0:Ch + 1, 0:Ch + 1])
                        rcp = rcp_pool.tile([128, 1], F32, tag="rcp",
                                            name="rcp")
                        nc.vector.reciprocal(rcp[:], tp[:, Ch:Ch + 1])
                        o = o_pool.tile([128, Ch], F32, tag="o", name="o")
                        nc.vector.tensor_scalar_mul(o[:], tp[:, 0:Ch], rcp[:])
                        nc.sync.dma_start(
                            out_d[bi, hi, l0 + jj * 128:l0 + (jj + 1) * 128, :],
                            o[:])
    nc.compile()
    return nc


_NC_CACHE = None


def _get_nc():
    global _NC_CACHE
    if _NC_CACHE is None:
        _NC_CACHE = _build_nc()
    return _NC_CACHE


def _make_in_maps(x, wq, bq, wk, bk, wv, bv):
    in_maps = []
    for core in range(N_CORES):
        bg, hg = divmod(core, 4)
        bs = [2 * bg, 2 * bg + 1]
        hs = [2 * hg, 2 * hg + 1]

        def packw(w):
            return np.concatenate(
                [w[h * Ch:(h + 1) * Ch].T for h in hs], axis=1)

        def packb(b):
            return np.concatenate([b[h * Ch:(h + 1) * Ch] for h in hs])

        in_maps.append({
            "x": np.ascontiguousarray(x[bs]),
            "w": np.ascontiguousarray(
                np.stack([packw(wq), packw(wk), packw(wv)])),
            "b": np.ascontiguousarray(
                np.stack([packb(bq), packb(bk), packb(bv)])),
        })
    return in_maps


def kernel(x, wq, bq, wk, bk, wv, bv):
    global LAST_EXEC_NS
    from concourse.bass_utils import run_bass_kernel_spmd

    nc = _get_nc()
    in_maps = _make_in_maps(
        np.asarray(x, dtype=np.float32),
        np.asarray(wq, np.float32), np.asarray(bq, np.float32),
        np.asarray(wk, np.float32), np.asarray(bk, np.float32),
        np.asarray(wv, np.float32), np.asarray(bv, np.float32))

    trace = os.environ.get("BASS_KERNEL_TRACE", "0") == "1"
    kwargs = {}
    if trace:
        kwargs.update(trace=True, trace_cores=[0])
    res = run_bass_kernel_spmd(nc, in_maps, list(range(N_CORES)), **kwargs)
    LAST_EXEC_NS = res.exec_time_ns

    out = np.empty((B, C, L), dtype=np.float32)
    for core in range(N_CORES):
        bg, hg = divmod(core, 4)
        o = res.results[core]["out"]
        for bi in range(2):
            for hi in range(2):
                b_ = 2 * bg + bi
                h_ = 2 * hg + hi
                out[b_, h_ * Ch:(h_ + 1) * Ch, :] = o[bi, hi].reshape(Ch, L)
    return out


# revision 31
# speedup vs baseline: 1.3247x; 1.0340x over previous
import os
import sys
from contextlib import ExitStack

import numpy as np

if "/opt/trn_rl_repo" not in sys.path:
    sys.path.insert(0, "/opt/trn_rl_repo")

import concourse.bass as bass
import concourse.tile as tile
from concourse import bacc, mybir
from concourse.masks import make_identity

B, C, L = 4, 512, 2048
H, Ch = 8, 64
N_CORES = 8
KC = 4
NT = 4
LB = 2
MC = 16
F32 = mybir.dt.float32
F32R = mybir.dt.float32r
BF16 = mybir.dt.bfloat16

LAST_EXEC_NS = None


def _build_nc():
    nc = bacc.Bacc("TRN2", target_bir_lowering=False, debug=False,
                   num_devices=N_CORES)
    x_d = nc.dram_tensor("x", [2, C, L], F32, kind="ExternalInput").ap()
    w_d = nc.dram_tensor("w", [3, C, 128], F32, kind="ExternalInput").ap()
    b_d = nc.dram_tensor("b", [3, 128], F32, kind="ExternalInput").ap()
    out_d = nc.dram_tensor("out", [2, 2, L, Ch], F32, kind="ExternalOutput").ap()
    v_d = nc.dram_tensor("vdram", [2, 128, L], BF16).ap()

    with tile.TileContext(nc) as tc, ExitStack() as ctx:
        const = ctx.enter_context(tc.tile_pool(name="const", bufs=1))
        persist = ctx.enter_context(tc.tile_pool(name="persist", bufs=1))

        ident = const.tile([128, 128], F32, tag="ident", name="ident")
        make_identity(nc, ident[:])

        w_sb = const.tile([128, 3, KC, 128], F32R, tag="w", name="w")
        nc.sync.dma_start(w_sb[:], w_d.rearrange("g (kc p) m -> p g kc m",
                                                 p=128).bitcast(F32R))
        b_sb = const.tile([128, 3], F32, tag="b", name="b")
        nc.sync.dma_start(b_sb[:], b_d.rearrange("g p -> p g"))

        x_sb = {}
        for bi in range(2):
            x_sb[bi] = persist.tile([128, KC, L], F32R, tag=f"x{bi}",
                                    name=f"x{bi}")
            for kc in range(KC):
                nc.sync.dma_start(
                    x_sb[bi][:, kc, :],
                    x_d[bi, kc * 128:(kc + 1) * 128, :].bitcast(F32R))

        q_sb, k_sb, v_sb, v2_sb = {}, {}, {}, {}
        for bi in range(2):
            q_sb[bi] = persist.tile([128, L], BF16, tag=f"q{bi}", name=f"q{bi}")
            v_sb[bi] = persist.tile([128, L], BF16, tag=f"v{bi}", name=f"v{bi}")
            for hi in range(2):
                t = persist.tile([128, L], BF16, tag=f"k{bi}{hi}",
                                 name=f"k{bi}{hi}")
                nc.vector.memset(t[(1 - hi) * 64:(2 - hi) * 64, :], 0.0)
                k_sb[bi, hi] = t
        for bi in range(2):
            for hi in range(2):
                v2_sb[bi, hi] = persist.tile([128, MC, Ch + 1], BF16,
                                             tag=f"v2_{bi}{hi}",
                                             name=f"v2_{bi}{hi}")
                nc.vector.memset(v2_sb[bi, hi][:, :, Ch:Ch + 1], 1.0)

        proj_ps = ctx.enter_context(
            tc.tile_pool(name="proj_ps", bufs=2, space="PSUM"))

        def bias_copy(eng, dst_ap, src_ap, bias_ap):
            if eng == 0:
                nc.vector.tensor_scalar_add(dst_ap, src_ap, bias_ap)
            else:
                nc.scalar.add(dst_ap, src_ap, bias_ap)

        dst = [q_sb, None, v_sb]

        def proj_group(bi, g):
            for half in range(2):
                ps = [proj_ps.tile([128, 512], F32, tag="pp", name="pp")
                      for _ in range(2)]
                for kc in range(KC):
                    for j in range(2):
                        nt = half * 2 + j
                        nc.tensor.matmul(
                            ps[j][:],
                            w_sb[:, g, kc, :],
                            x_sb[bi][:, kc, nt * 512:(nt + 1) * 512],
                            start=(kc == 0), stop=(kc == KC - 1))
                for j in range(2):
                    nt = half * 2 + j
                    sl = slice(nt * 512, (nt + 1) * 512)
                    if g == 1:
                        for hi in range(2):
                            pr = slice(hi * 64, (hi + 1) * 64)
                            bias_copy(hi, k_sb[bi, hi][pr, sl],
                                      ps[j][pr, :], b_sb[pr, g:g + 1])
                    else:
                        bias_copy(j, dst[g][bi][:, sl], ps[j][:],
                                  b_sb[:, g:g + 1])
            if g == 2:
                nc.sync.dma_start(v_d[bi], v_sb[bi][:])
                for hi in range(2):
                    for mc in range(MC):
                        vsrc = v_d[bi,
                                   hi * 64 + 4 * mc:hi * 64 + 4 * mc + 4, :]
                        nc.sync.dma_start(
                            v2_sb[bi, hi][:, mc, 0:Ch],
                            vsrc.rearrange("a (j cc) -> (a j) cc",
                                           j=32, cc=Ch))

        s_pool = ctx.enter_context(
            tc.tile_pool(name="s_ps", bufs=2, space="PSUM"))
        av_pool = ctx.enter_context(
            tc.tile_pool(name="av_ps", bufs=1, space="PSUM"))
        pt_pool = ctx.enter_context(tc.tile_pool(name="pt", bufs=6))
        avs_pool = ctx.enter_context(tc.tile_pool(name="avs", bufs=2))
        rcp_pool = ctx.enter_context(tc.tile_pool(name="rcp", bufs=4))
        o_pool = ctx.enter_context(tc.tile_pool(name="o", bufs=4))

        def ablock(bi, hi, lb):
            l0 = lb * 1024
            av = av_pool.tile([Ch + 1, 1024], F32, tag="av", name="av")
            for mc in range(MC):
                s = s_pool.tile([128, 1024], F32, tag="s", name="s")
                for n2 in range(2):
                    nc.tensor.matmul(
                        s[:, n2 * 512:(n2 + 1) * 512],
                        k_sb[bi, hi][:, mc * 128:(mc + 1) * 128],
                        q_sb[bi][:, l0 + n2 * 512:l0 + (n2 + 1) * 512],
                        start=True, stop=True)
                pt = pt_pool.tile([128, 1024], BF16, tag="pt", name="pt")
                nc.scalar.activation(pt[:], s[:],
                                     mybir.ActivationFunctionType.Exp,
                                     scale=0.125)
                for n2 in range(2):
                    nc.tensor.matmul(
                        av[:, n2 * 512:(n2 + 1) * 512],
                        v2_sb[bi, hi][:, mc, :],
                        pt[:, n2 * 512:(n2 + 1) * 512],
                        start=(mc == 0), stop=(mc == MC - 1))

            def tail():
                avs = avs_pool.tile([Ch + 1, 1024], F32, tag="avs",
                                    name="avs")
                nc.vector.tensor_copy(avs[:], av[:])
                for jj in range(8):
                    tp = proj_ps.tile([128, Ch + 1], F32, tag="pp",
                                      name="tp")
                    nc.tensor.transpose(
                        tp[:], avs[:, jj * 128:(jj + 1) * 128],
                        ident[0:Ch + 1, 0:Ch + 1])
                    rcp = rcp_pool.tile([128, 1], F32, tag="rcp", name="rcp")
                    nc.vector.reciprocal(rcp[:], tp[:, Ch:Ch + 1])
                    o = o_pool.tile([128, Ch], F32, tag="o", name="o")
                    nc.vector.tensor_scalar_mul(o[:], tp[:, 0:Ch], rcp[:])
                    nc.sync.dma_start(
                        out_d[bi, hi, l0 + jj * 128:l0 + (jj + 1) * 128, :],
                        o[:])
            return tail

        blocks = [(0, 0, 0), (0, 0, 1), (0, 1, 0), (0, 1, 1),
                  (1, 0, 0), (1, 0, 1), (1, 1, 0), (1, 1, 1)]
        between = {0: lambda: proj_group(1, 2), 1: lambda: proj_group(1, 1),
                   2: lambda: proj_group(1, 0)}
        for g in (2, 1, 0):
            proj_group(0, g)
        prev_tail = None
        for i, (bi, hi, lb) in enumerate(blocks):
            t = ablock(bi, hi, lb)
            if prev_tail is not None:
                prev_tail()
            prev_tail = t
            if i in between:
                between[i]()
        prev_tail()
    nc.compile()
    return nc
